# revision 3
# baseline (speedup 1.0000x reference)
"""Causal self-attention Trainium2 kernel.

Problem: B=2, T=2048, C=1024, H=16 heads (head_dim 64) causal attention
(x @ Wq/Wk/Wv -> heads -> softmax(q k^T / sqrt(64)) @ v -> @ Wp).

Sharding over 8 NeuronCores: core c handles batch b = c // 4 and the 4
heads h in [4*(c%4), 4*(c%4)+4)  (channel slice of 256 columns of the
QKV weights / 256 rows of Wp).  Each core computes a partial output
y_part @ Wp_slice of shape [T, C]; the host sums the 4 partials per
batch and adds the bias term (bv @ Wp + bp), which is exact because
softmax rows sum to 1 so the v-bias passes through attention additively.

On-device layout (per core):
  xT  [C, T]   (host-pretransposed)  -> SBUF [128, 8, T]
  qT/kT = W^T x^T computed directly in [ch, T] layout so that the
  attention score matmul  ST[tk, tq] = sum_d k[tk,d] q[tq,d]  uses
  lhsT = kT (d on partitions), rhs = qT.  Head pairs are packed into
  the 128-partition dim (d=64 each) -> two concurrent row-group matmuls.
  Softmax runs on ST in [tk(P), tq(free)] orientation: causal masking by
  shrinking the tq window per diagonal tile plus one additive -1e30
  [128,128] mask on PSUM before the exp; exp on ACT (scale=1/8 fused).
  y^T[d, tq] accumulates in PSUM via lhsT = [v | ones] (65 cols), so
  row 64 of the accumulator is the softmax denominator; normalization
  is a DVE reciprocal + a K=1 matmul broadcast + DVE multiply.

All matmul operands are float32r (fp32 bits, reduced-precision PE path):
1 cycle/row when the moving free dim is >= 256 vs 4 cycles/row for fp32.
Measured operand rounding error ~2e-4 relative.
"""

import os
import sys

import numpy as np

for _p in ("/opt/trn_rl_repo", "/root/.axon_site/_ro/trn_rl_repo"):
    if _p not in sys.path and os.path.isdir(_p):
        sys.path.insert(0, _p)

import concourse.bacc as bacc  # noqa: E402
import concourse.mybir as mybir  # noqa: E402
import concourse.tile as tile  # noqa: E402
from concourse.bass_utils import run_bass_kernel_spmd  # noqa: E402

P = 128
B, T, C, H = 2, 2048, 1024, 16
HD = 64
HG = 4          # heads per core
CS = HG * HD    # channel slice per core = 256
NKC = C // P    # 8 contraction tiles over C
NTT = T // P    # 16 token tiles
TQ = 512        # query-block width
NTQ = T // TQ   # 4
F32 = mybir.dt.float32
F32R = mybir.dt.float32r
EXPF = mybir.ActivationFunctionType.Exp
IDENT = mybir.ActivationFunctionType.Identity


def build_nc():
    nc = bacc.Bacc("TRN2")

    xT_d = nc.dram_tensor("xT", [C, T], F32R, kind="ExternalInput")
    wq_d = nc.dram_tensor("wq", [C, CS], F32R, kind="ExternalInput")
    wk_d = nc.dram_tensor("wk", [C, CS], F32R, kind="ExternalInput")
    wv_d = nc.dram_tensor("wv", [C, CS], F32R, kind="ExternalInput")
    wp_d = nc.dram_tensor("wp", [CS, C], F32R, kind="ExternalInput")
    bq_d = nc.dram_tensor("bq", [CS], F32, kind="ExternalInput")
    bk_d = nc.dram_tensor("bk", [CS], F32, kind="ExternalInput")
    mn_d = nc.dram_tensor("mneg", [P, P], F32, kind="ExternalInput")
    on_d = nc.dram_tensor("onesv", [P, P], F32R, kind="ExternalInput")
    out_d = nc.dram_tensor("out", [T, C], F32, kind="ExternalOutput")

    with tile.TileContext(nc) as tc:
        with (
            tc.tile_pool(name="big", bufs=1) as big,
            tc.tile_pool(name="const", bufs=1) as const,
            tc.tile_pool(name="work", bufs=6) as work,
            tc.tile_pool(name="ost", bufs=4) as ostp,
            tc.tile_pool(name="psA", bufs=2, space="PSUM") as psA,
            tc.tile_pool(name="psS", bufs=3, space="PSUM") as psS,
            tc.tile_pool(name="psY", bufs=3, space="PSUM") as psY,
        ):
            # ---- persistent SBUF tensors ----
            xT = big.tile([P, NKC, T], F32R, tag="xT")
            qT = big.tile([P, 2, T], F32R, tag="qT")
            kT = big.tile([P, 2, T], F32R, tag="kT")
            yT = big.tile([P, 2, T], F32R, tag="yT")
            vx = big.tile([P, NTT, HG, 66], F32R, tag="vx")  # [v | 1] per head
            wq = const.tile([P, NKC, CS], F32R, tag="wq")
            wk = const.tile([P, NKC, CS], F32R, tag="wk")
            wv = const.tile([P, NKC, CS], F32R, tag="wv")
            wp = const.tile([P, CS // P, C], F32R, tag="wp")
            bq = const.tile([P, 2], F32, tag="bq")
            bk = const.tile([P, 2], F32, tag="bk")
            mneg = const.tile([P, P], F32, tag="mneg")
            ones1 = const.tile([1, HD], F32R, tag="ones1")

            # ---- loads (split across DMA queues) ----
            with nc.named_scope("load"):
                xT3 = xT_d.ap().rearrange("(o p) t -> p o t", p=P)
                for o in range(NKC):
                    nc.sync.dma_start(xT[:, o, :], xT3[:, o, :])
                for w_sb, w_d in ((wq, wq_d), (wk, wk_d), (wv, wv_d)):
                    w3 = w_d.ap().rearrange("(o p) n -> p o n", p=P)
                    for o in range(0, NKC, 2):
                        nc.sync.dma_start(w_sb[:, o:o + 2, :], w3[:, o:o + 2, :])
                wp3 = wp_d.ap().rearrange("(o p) n -> p o n", p=P)
                for o in range(CS // P):
                    nc.sync.dma_start(wp[:, o, :], wp3[:, o, :])
                nc.sync.dma_start(bq[:], bq_d.ap().rearrange("(o p) -> p o", p=P))
                nc.sync.dma_start(bk[:], bk_d.ap().rearrange("(o p) -> p o", p=P))
                nc.sync.dma_start(mneg[:], mn_d.ap())
                nc.sync.dma_start(ones1[:], on_d.ap()[0:1, 0:HD])
                nc.sync.dma_start(
                    vx[:, :, :, 64:66],
                    on_d.ap().rearrange("p (t h c) -> p t h c", t=NTT, h=HG),
                )

            # ---- phase 1: qT, kT  ([ch, T] = W^T @ x^T) ----
            with nc.named_scope("qk"):
                for dst, w_sb, b_sb in ((qT, wq, bq), (kT, wk, bk)):
                    for cht in range(2):
                        for tqi in range(NTQ):
                            ps = psA.tile([P, TQ], F32, tag="mm", name="ps_qk")
                            for kc in range(NKC):
                                nc.tensor.matmul(
                                    ps[:],
                                    lhsT=w_sb[:, kc, cht * P:(cht + 1) * P],
                                    rhs=xT[:, kc, tqi * TQ:(tqi + 1) * TQ],
                                    start=(kc == 0),
                                    stop=(kc == NKC - 1),
                                )
                            # psum -> sbuf with per-channel bias add
                            nc.scalar.activation(
                                dst[:, cht, tqi * TQ:(tqi + 1) * TQ],
                                ps[:], IDENT, bias=b_sb[:, cht:cht + 1],
                            )

            # ---- phase 2: v ([t, d] natural layout, all 4 heads) ----
            with nc.named_scope("v"):
                for tt in range(NTT):
                    ps = psA.tile([P, CS], F32, tag="mm", name="ps_v")
                    for kc in range(NKC):
                        nc.tensor.matmul(
                            ps[:],
                            lhsT=xT[:, kc, tt * P:(tt + 1) * P],
                            rhs=wv[:, kc, :],
                            start=(kc == 0),
                            stop=(kc == NKC - 1),
                        )
                    nc.vector.tensor_copy(
                        out=vx[:, tt, :, 0:64],
                        in_=ps[:].rearrange("p (h d) -> p h d", h=HG),
                    )

            # ---- phase 3: attention, head-pairs packed in partition dim ----
            with nc.named_scope("attn"):
                for pair in range(2):          # cht: heads (2*pair, 2*pair+1)
                    for tqi in range(NTQ):
                        tq0 = tqi * TQ
                        ntk = (tq0 + TQ) // P
                        y_ps = [psY.tile([P, TQ], F32, tag="y", name=f"y{h2}")
                                for h2 in range(2)]
                        est_q = []  # pipelined: (tk, [(d_off, ncols, est)*2])
                        for tk in range(ntk + 1):
                            if tk < ntk:
                                tk0 = tk * P
                                d_off = max(0, tk0 - tq0)
                                ncols = TQ - d_off
                                ests = []
                                for h2 in range(2):
                                    prow = slice(h2 * HD, (h2 + 1) * HD)
                                    st = psS.tile([P, TQ], F32, tag="st",
                                                  name="st")
                                    nc.tensor.matmul(
                                        st[:, :ncols],
                                        lhsT=kT[prow, pair, tk0:tk0 + P],
                                        rhs=qT[prow, pair,
                                               tq0 + d_off:tq0 + TQ],
                                        start=True, stop=True,
                                    )
                                    if tk0 >= tq0:  # diagonal: additive mask
                                        nc.vector.tensor_add(
                                            out=st[:, :P], in0=st[:, :P],
                                            in1=mneg[:],
                                        )
                                    est = work.tile([P, TQ], F32R, tag="est",
                                                    name="est")
                                    nc.scalar.activation(
                                        est[:, :ncols], st[:, :ncols],
                                        EXPF, scale=0.125,
                                    )
                                    ests.append((d_off, ncols, est))
                                est_q.append((tk, ests))
                            if est_q and (tk >= 1 or tk == ntk):
                                jtk, ests = est_q.pop(0)
                                for h2 in range(2):
                                    d_off, ncols, est = ests[h2]
                                    h = pair * 2 + h2
                                    nc.tensor.matmul(
                                        y_ps[h2][:HD + 1, d_off:TQ],
                                        lhsT=vx[:, jtk, h, 0:HD + 1],
                                        rhs=est[:, :ncols],
                                        start=(jtk == 0),
                                        stop=(jtk == ntk - 1),
                                        skip_group_check=True,
                                    )
                        assert not est_q
                        # normalize: r = 1/rowsum, broadcast via K=1 matmul
                        for h2 in range(2):
                            prow = slice(h2 * HD, (h2 + 1) * HD)
                            r_sb = work.tile([1, TQ], F32R, tag="r", name="r")
                            with nc.allow_low_precision(
                                    reason="f32r keeps full fp32 bits"):
                                nc.vector.reciprocal(
                                    r_sb[:], y_ps[h2][HD:HD + 1, :])
                            rb = psS.tile([HD, TQ], F32, tag="st", name="rb")
                            nc.tensor.matmul(rb[:], lhsT=ones1[:], rhs=r_sb[:],
                                             start=True, stop=True)
                            ytmp = work.tile([HD, TQ], F32, tag="ytmp",
                                             name="ytmp")
                            nc.scalar.copy(ytmp[:], y_ps[h2][:HD, :])
                            nc.vector.tensor_mul(
                                out=yT[prow, pair, tq0:tq0 + TQ],
                                in0=ytmp[:], in1=rb[:],
                            )

            # ---- phase 4: output projection (partial: y_part @ Wp_slice) ----
            with nc.named_scope("proj"):
                for tt in range(NTT):
                    for co in range(2):
                        ps = psA.tile([P, TQ], F32, tag="mm", name="ps_proj")
                        for cht in range(CS // P):
                            nc.tensor.matmul(
                                ps[:],
                                lhsT=yT[:, cht, tt * P:(tt + 1) * P],
                                rhs=wp[:, cht, co * TQ:(co + 1) * TQ],
                                start=(cht == 0),
                                stop=(cht == CS // P - 1),
                            )
                        ost = ostp.tile([P, TQ], F32, tag="ost", name="ost")
                        nc.vector.tensor_copy(out=ost[:], in_=ps[:])
                        nc.sync.dma_start(
                            out_d.ap()[tt * P:(tt + 1) * P,
                                       co * TQ:(co + 1) * TQ],
                            ost[:],
                        )

    nc.compile()
    return nc


_NC = None


def _get_nc():
    global _NC
    if _NC is None:
        _NC = build_nc()
    return _NC


def _make_in_maps(x, Wq, bq, Wk, bk, Wv, bv, Wp, bp):
    f = lambda a: np.ascontiguousarray(np.asarray(a, dtype=np.float32))
    x, Wq, Wk, Wv, Wp = f(x), f(Wq), f(Wk), f(Wv), f(Wp)
    bq, bk = f(bq), f(bk)
    xT = [np.ascontiguousarray(x[b].T) for b in range(B)]
    triu = np.triu(np.ones((P, P), dtype=np.float32))
    mneg = np.where(triu > 0, 0.0, -1e30).astype(np.float32)
    onesv = np.ones((P, P), dtype=np.float32)
    in_maps = []
    for c in range(8):
        b, g = divmod(c, 4)
        cs = slice(g * CS, (g + 1) * CS)
        in_maps.append({
            "xT": xT[b],
            "wq": np.ascontiguousarray(Wq[:, cs]),
            "wk": np.ascontiguousarray(Wk[:, cs]),
            "wv": np.ascontiguousarray(Wv[:, cs]),
            "wp": np.ascontiguousarray(Wp[cs, :]),
            "bq": np.ascontiguousarray(bq[cs]),
            "bk": np.ascontiguousarray(bk[cs]),
            "mneg": mneg,
            "onesv": onesv,
        })
    return in_maps


def _assemble(results, bias_term):
    out = np.empty((B, T, C), dtype=np.float32)
    for b in range(B):
        acc = results[4 * b]["out"].astype(np.float32, copy=True)
        for g in range(1, 4):
            acc += results[4 * b + g]["out"]
        out[b] = acc + bias_term
    return out


def kernel(x, Wq, bq, Wk, bk, Wv, bv, Wp, bp):
    nc = _get_nc()
    in_maps = _make_in_maps(x, Wq, bq, Wk, bk, Wv, bv, Wp, bp)
    res = run_bass_kernel_spmd(nc, in_maps, core_ids=list(range(8)))
    bias_term = (np.asarray(bv, np.float32) @ np.asarray(Wp, np.float32)
                 + np.asarray(bp, np.float32)).astype(np.float32)
    return _assemble(res.results, bias_term)


if __name__ == "__main__":
    rng = np.random.default_rng(0)
    s = 1.0 / np.sqrt(C)
    inputs = {
        "x": rng.standard_normal((B, T, C), dtype=np.float32),
        "Wq": rng.standard_normal((C, C), dtype=np.float32) * s,
        "bq": np.zeros(C, np.float32),
        "Wk": rng.standard_normal((C, C), dtype=np.float32) * s,
        "bk": np.zeros(C, np.float32),
        "Wv": rng.standard_normal((C, C), dtype=np.float32) * s,
        "bv": np.zeros(C, np.float32),
        "Wp": rng.standard_normal((C, C), dtype=np.float32) * s,
        "bp": np.zeros(C, np.float32),
    }
    out = kernel(**inputs)
    print("out", out.shape, out.dtype, float(np.abs(out).max()))
